# revision 1
# baseline (speedup 1.0000x reference)
"""Causal self-attention (B=4, T=2048, C=1024, H=16, HD=64) on 8 trn2 cores.

Sharding: core = (batch b, head-group g), g in {0,1} covering 8 heads each.
Each core: QKV projection for its 8 heads, causal attention, partial output
projection y_g @ W_proj[g*512:(g+1)*512]. Host sums the two partials and adds
the output bias (which absorbs b_v: softmax weights sum to 1, so the V bias
passes through attention exactly and b_eff = b_proj + b_v @ W_proj).

Quantization recipe (max rel err vs f32 reference ~8e-3, gate 2e-2):
  - QK projection: fp8e4 x and W (W pre-scaled x16 on host to stay in fp8
    normals; un-scaled in the PSUM->SBUF copy), DoubleRow matmuls: the HW
    streams 1 col/cycle regardless, but DoubleRow doubles the contraction
    per pass (256), halving instruction count. Q/K stored bf16.
  - S = K^T Q in bf16 (64-deep row-group pairs run concurrently on the PE).
  - P = exp(S/8) in bf16 (ACT); causal masking = lower-triangle multiply on
    the bf16 tile (DVE, 2x mode) for diagonal blocks.
  - AV in bf16 (same col count as fp8-DoubleRow+residual would need, with
    none of the fp8 error). The ones column of V_aug makes PSUM row 64 the
    softmax denominator.
  - y normalized via gpsimd partition-broadcast + fast reciprocal, stored
    bf16; output projection in bf16.

Scheduling: emit_b is software-pipelined at kc granularity (S of kc+1 is
emitted before the AV that waits on exp(kc), so the PE never head-of-line
blocks on the ACT engine); V projections, output projections and the n=3 QK
groups fill PE slack during ACT-bound attention stretches; x streams in
T-major chunks; QK PSUM->SBUF copies run on ACT (idle during phase A).
"""

import numpy as np

B, T, C, H, HD = 4, 2048, 1024, 16, 64
G = 2              # head groups (tensor parallel)
HG = H // G        # 8 heads per group
GC = HG * HD       # 512 group channels
P = 128
NQC = T // 512     # 4 q-chunks of 512
NKC = T // P       # 16 k-chunks of 128
KO_C = C // P      # 8 contraction chunks for C=1024
KO_G = GC // P     # 4 contraction chunks for GC=512
WSCALE = 16.0      # host-side premultiplier on W_attn qk columns (fp8 normals)

_cache = {}


def _build():
    import concourse.bass as bass
    import concourse.tile as tile
    from concourse import bacc, mybir

    f32 = mybir.dt.float32
    f32r = mybir.dt.float32r
    bf16 = mybir.dt.bfloat16
    fp8 = mybir.dt.float8e4
    DR = mybir.MatmulPerfMode.DoubleRow

    nc = bacc.Bacc(name="csa")
    x8 = nc.declare_dram_parameter("x8", [P, KO_C, T], fp8, isOutput=False)
    xbf = nc.declare_dram_parameter("xbf", [P, KO_C, T], bf16, isOutput=False)
    wqk = nc.declare_dram_parameter("wqk", [2 * GC // P, P, KO_C, P], fp8, isOutput=False)
    bqk = nc.declare_dram_parameter("bqk", [P, 2 * GC // P], f32, isOutput=False)
    wv = nc.declare_dram_parameter("wv", [P, KO_C, GC], bf16, isOutput=False)
    wp = nc.declare_dram_parameter("wp", [P, KO_G, C], bf16, isOutput=False)
    mask = nc.declare_dram_parameter("mask", [P, P], bf16, isOutput=False)
    out = nc.declare_dram_parameter("out", [T, C], f32, isOutput=True)

    from contextlib import ExitStack

    with tile.TileContext(nc) as tc, ExitStack() as ctx:
            singles = ctx.enter_context(tc.tile_pool(name="singles", bufs=1))
            ppool = ctx.enter_context(tc.tile_pool(name="ppool", bufs=3))
            spool = ctx.enter_context(tc.tile_pool(name="spool", bufs=2))
            pp = ctx.enter_context(tc.tile_pool(name="pp", bufs=2, space="PSUM"))
            ps = ctx.enter_context(tc.tile_pool(name="ps", bufs=2, space="PSUM"))
            py = ctx.enter_context(tc.tile_pool(name="py", bufs=2, space="PSUM"))
            # ---- resident tensors ----
            xbf_s = singles.tile([P, KO_C, T], bf16, tag="xbf")   # V-proj lhsT
            x8_s = singles.tile([P, KO_C, T], fp8, tag="x8")      # QK-proj rhs
            wqk_s = singles.tile([P, 2 * GC // P, KO_C, P], fp8, tag="wqk")

            # ko-chunked full-T transfers: per-partition lines are 2KB (x8) /
            # 4KB (xbf), vs 512B for T-chunked slices (which ran at ~50GB/s)
            d_prev = None
            for _ko in range(KO_C):
                d_prev = nc.sync.dma_start(out=x8_s[:, _ko, :], in_=x8[:, _ko, :])
            for _m in range(2 * GC // P):
                d_wqk = nc.sync.dma_start(out=wqk_s[:, _m], in_=wqk[_m])
                tile.add_dep_helper(d_wqk.ins, d_prev.ins, reason="dma order")
            d_prev = d_wqk
            for _ko in range(KO_C):
                d2 = nc.sync.dma_start(out=xbf_s[:, _ko, :], in_=xbf[:, _ko, :])
                tile.add_dep_helper(d2.ins, d_prev.ins, reason="dma order")
            d_prev = d2
            wv_s = singles.tile([P, KO_C, GC], bf16, tag="wv")
            for _ko in range(KO_C):
                d2 = nc.sync.dma_start(out=wv_s[:, _ko, :], in_=wv[:, _ko, :])
                tile.add_dep_helper(d2.ins, d_prev.ins, reason="dma order")
            d_prev = d2
            wp_s = singles.tile([P, KO_G, C], bf16, tag="wp")
            for _ko in range(KO_G):
                _d = nc.sync.dma_start(out=wp_s[:, _ko, :], in_=wp[:, _ko, :])
                tile.add_dep_helper(_d.ins, d_prev.ins, reason="dma order")

            QT = singles.tile([P, HG // 2, T], bf16, tag="QT")
            KT = singles.tile([P, HG // 2, T], bf16, tag="KT")
            # V augmented bf16: cols 0..63 = V, col 64 = ones (softmax
            # denominator)
            vaug = singles.tile([P, NKC, HG, 65], bf16, tag="vaug")
            ones_sb = singles.tile([P, 1], f32, tag="ones_sb")
            nc.vector.memset(ones_sb[:], 1.0)
            nc.vector.tensor_copy(
                out=vaug[:, :, :, 64:65],
                in_=ones_sb[:, :, None, None].to_broadcast((P, NKC, HG, 1)),
            )
            tri = singles.tile([P, P], bf16, tag="tri")
            nc.sync.dma_start(out=tri[:], in_=mask[:])
            bqk_s = singles.tile([P, 2 * GC // P], f32, tag="bqk")
            nc.sync.dma_start(out=bqk_s[:], in_=bqk[:])

            # ---- QK projection: fp8 DoubleRow, contraction 256/instr ----
            def emit_qk_group(m, n):
                acc = pp.tile([P, 512], f32, tag="pp")
                for k2 in range(KO_C // 2):
                    nc.tensor.matmul(
                        acc[:],
                        lhsT=wqk_s[:, m, 2 * k2:2 * k2 + 2, :],
                        rhs=x8_s[:, 2 * k2:2 * k2 + 2, n * 512:(n + 1) * 512],
                        start=(k2 == 0),
                        stop=(k2 == KO_C // 2 - 1),
                        perf_mode=DR,
                    )
                dest = QT if m < 4 else KT
                # (acc/WSCALE) + bias -> bf16, on ACT (idle in phase A)
                nc.scalar.activation(
                    dest[:, m % 4, n * 512:(n + 1) * 512], acc[:],
                    mybir.ActivationFunctionType.Identity,
                    bias=bqk_s[:, m:m + 1], scale=1.0 / WSCALE,
                )

            # y^T aliases the first 4 ko-chunks of xbf (x cols there are dead
            # once the V projection for the matching t-chunks is done; QK proj
            # reads the separate x8 copy, so no race even for qc=0)
            YT = xbf_s[:, 0:KO_G, :]

            # ---- V-projection / output-projection emitters ----
            def emit_v(t):
                acc = pp.tile([P, GC], f32, tag="pp")
                for ko in range(KO_C):
                    nc.tensor.matmul(
                        acc[:],
                        lhsT=xbf_s[:, ko, t * P:(t + 1) * P],
                        rhs=wv_s[:, ko, :],
                        start=(ko == 0),
                        stop=(ko == KO_C - 1),
                    )
                nc.vector.tensor_copy(
                    out=vaug[:, t, :, 0:64],
                    in_=acc[:].rearrange("p (h d) -> p h d", h=HG),
                )

            def emit_c(t, n):
                opsum = pp.tile([P, 512], f32, tag="pp")
                for ko in range(KO_G):
                    nc.tensor.matmul(
                        opsum[:],
                        lhsT=YT[:, ko, t * P:(t + 1) * P],
                        rhs=wp_s[:, ko, n * 512:(n + 1) * 512],
                        start=(ko == 0),
                        stop=(ko == KO_G - 1),
                    )
                osb = ppool.tile([P, 512], f32, tag="osb")
                nc.vector.tensor_copy(out=osb[:], in_=opsum[:])
                nc.sync.dma_start(
                    out=out[t * P:(t + 1) * P, n * 512:(n + 1) * 512],
                    in_=osb[:],
                )

            # ---- attention for one (q-chunk, head-pair) ----
            # software-pipelined: S(kc+1) is emitted before the AV of kc, so
            # when the AV waits on exp the PE keeps streaming S.
            def emit_b(qc, hp):
                    nkc = 4 * (qc + 1)
                    qo_of = [max(kc - 4 * qc, 0) * P for kc in range(nkc)]
                    ype = py.tile([P, 512], f32, tag="py")
                    ypo = py.tile([P, 512], f32, tag="py")
                    spsums = {}
                    pts = {}

                    def emit_s(kc):
                        qo = qo_of[kc]
                        w = 512 - qo
                        spsum = ps.tile([P, 2, 512], f32, tag="ps")
                        spsums[kc] = spsum
                        for odd in (0, 1):
                            po = odd * 64
                            nc.tensor.matmul(
                                spsum[:, odd, 0:w],
                                lhsT=KT[po:po + 64, hp, kc * P:(kc + 1) * P],
                                rhs=QT[po:po + 64, hp,
                                       qc * 512 + qo:(qc + 1) * 512],
                                start=True,
                                stop=True,
                            )

                    def emit_exp(kc):
                        qo = qo_of[kc]
                        w = 512 - qo
                        pt = ppool.tile([P, 2, 512], bf16, tag="pt")
                        pts[kc] = pt
                        nc.scalar.activation(
                            pt[:, :, 0:w], spsums.pop(kc)[:, :, 0:w],
                            mybir.ActivationFunctionType.Exp, scale=0.125,
                        )
                        if kc >= 4 * qc:
                            nc.vector.tensor_tensor(
                                pt[:, :, 0:P], pt[:, :, 0:P],
                                tri[:, None, :].to_broadcast((P, 2, P)),
                                mybir.AluOpType.mult,
                            )

                    def emit_av(kc):
                        pt = pts.pop(kc)
                        qo = qo_of[kc]
                        w = 512 - qo
                        for odd, yp in ((0, ype), (1, ypo)):
                            nc.tensor.matmul(
                                yp[0:65, qo:512],
                                lhsT=vaug[:, kc, 2 * hp + odd, :],
                                rhs=pt[:, odd, 0:w],
                                start=(kc == 0),
                                stop=(kc == nkc - 1),
                            )

                    emit_s(0)
                    for kc in range(nkc):
                        emit_exp(kc)
                        if kc + 1 < nkc:
                            emit_s(kc + 1)
                        emit_av(kc)
                    for odd, yp in ((0, ype), (1, ypo)):
                        po = odd * 64
                        # copy the PSUM out fast so the bank frees for the
                        # next head-pair; normalize from the SBUF copy
                        sum_sb = ppool.tile([1, 512], f32, tag="osb")
                        ycop = spool.tile([64, 512], f32, tag="ycop")
                        nc.vector.tensor_copy(out=sum_sb[:], in_=yp[64:65, :])
                        nc.vector.tensor_copy(out=ycop[:], in_=yp[0:64, :])
                        srep = spool.tile([64, 512], f32, tag="srep")
                        nc.gpsimd.partition_broadcast(srep[:], sum_sb[:])
                        nc.vector.reciprocal_approx_fast(out=srep[:], in_=srep[:])
                        yslice = YT[po:po + 64, hp, qc * 512:(qc + 1) * 512]
                        if odd == 0:
                            nc.vector.tensor_tensor(
                                yslice, ycop[:], srep[:], mybir.AluOpType.mult
                            )
                        else:
                            # DVE lanes can't shift partitions; stage at 0..63
                            # and DMA to partitions 64..127
                            yt_tmp = ppool.tile([64, 512], bf16, tag="ytmp")
                            nc.vector.tensor_tensor(
                                yt_tmp[:], ycop[:], srep[:], mybir.AluOpType.mult
                            )
                            nc.sync.dma_start(out=yslice, in_=yt_tmp[:])

            # ---- schedule ----
            # n-major QK sweep (each group is only 4 DoubleRow matmuls), with
            # V projections and qc=0 attention interleaved as data lands.
            for n in range(3):
                for hp in range(4):
                    emit_qk_group(hp, n)       # Q chunk hp, T-slice n
                    emit_qk_group(4 + hp, n)   # K chunk hp, T-slice n
                    if n == 0 and hp == 1:
                        for t in range(4):
                            emit_v(t)
                    if n == 0 and hp >= 2:
                        emit_b(0, hp - 2)      # needs QT/KT pair hp-2 @ n=0
                    if n == 1:
                        if hp == 0:
                            emit_b(0, 2)
                        elif hp == 1:
                            emit_b(0, 3)
                        else:
                            emit_v(2 + hp)     # t = 4, 5
                    if n == 2 and hp < 2:
                        emit_v(6 + hp)         # t = 6, 7
            # remaining QK groups (n=3) interleave with qc=1 attention
            for qc in range(1, NQC):
                for hp in range(HG // 2):
                    if qc == 1:
                        emit_qk_group(hp, 3)
                        emit_qk_group(4 + hp, 3)
                    if qc < NQC - 1:
                        emit_v(4 * (qc + 1) + hp)
                    t = (qc - 1) * 4 + hp
                    emit_c(t, 0)
                    emit_b(qc, hp)
                    emit_c(t, 1)
            # trailing output projection for the last q-chunk
            for t in range(12, 16):
                emit_c(t, 0)
                emit_c(t, 1)
    nc.finalize()
    return nc


def _get_nc():
    if "nc" not in _cache:
        _cache["nc"] = _build()
    return _cache["nc"]


def _prep_inputs(x, W_attn, b_attn, W_proj):
    import ml_dtypes

    bfloat16 = ml_dtypes.bfloat16
    f8 = ml_dtypes.float8_e4m3
    x = np.ascontiguousarray(np.asarray(x, np.float32))
    W_attn = np.asarray(W_attn, np.float32)
    b_attn = np.asarray(b_attn, np.float32)
    W_proj = np.asarray(W_proj, np.float32)
    mask = (np.arange(P)[:, None] <= np.arange(P)[None, :]).astype(np.float32)
    in_maps = []
    for b in range(B):
        xTb = np.ascontiguousarray(x[b].T.reshape(KO_C, P, T).transpose(1, 0, 2))
        x8b = np.clip(xTb, -240, 240).astype(f8)
        xbfb = xTb.astype(bfloat16)
        for g in range(G):
            qs, ks, vs = g * GC, C + g * GC, 2 * C + g * GC
            w2 = np.concatenate([W_attn[:, qs:qs + GC], W_attn[:, ks:ks + GC]], 1)
            in_maps.append({
                "x8": x8b,
                "xbf": xbfb,
                "wqk": np.ascontiguousarray(
                    (w2 * WSCALE).reshape(KO_C, P, 2 * GC // P, P)
                    .transpose(2, 1, 0, 3)).astype(f8),
                "bqk": np.ascontiguousarray(
                    np.concatenate([b_attn[qs:qs + GC], b_attn[ks:ks + GC]])
                    .reshape(2 * GC // P, P).T),
                "wv": np.ascontiguousarray(
                    W_attn[:, vs:vs + GC].reshape(KO_C, P, GC)
                    .transpose(1, 0, 2)).astype(bfloat16),
                "wp": np.ascontiguousarray(
                    W_proj[g * GC:(g + 1) * GC, :].reshape(KO_G, P, C)
                    .transpose(1, 0, 2)).astype(bfloat16),
                "mask": mask.astype(bfloat16),
            })
    return in_maps


def _run(inputs, trace=False):
    from concourse.bass_utils import run_bass_kernel_spmd

    nc = _get_nc()
    in_maps = _prep_inputs(
        inputs["x"], inputs["W_attn"], inputs["b_attn"], inputs["W_proj"]
    )
    res = run_bass_kernel_spmd(nc, in_maps, list(range(B * G)), trace=trace)
    W_proj_f = np.asarray(inputs["W_proj"], np.float32)
    b_attn_f = np.asarray(inputs["b_attn"], np.float32)
    # b_v passes through softmax exactly (weights sum to 1): fold it into the
    # output bias instead of adding it to V in the kernel
    b_eff = (np.asarray(inputs["b_proj"], np.float32)
             + b_attn_f[2 * C:] @ W_proj_f)
    outs = [
        res.results[2 * b]["out"] + res.results[2 * b + 1]["out"] + b_eff
        for b in range(B)
    ]
    return np.stack(outs).astype(np.float32), res


def kernel(**inputs):
    return _run(inputs, trace=False)[0]


if __name__ == "__main__":
    rng = np.random.default_rng(0)
    ins = {
        "x": rng.standard_normal((B, T, C), np.float32),
        "W_attn": rng.uniform(-0.03, 0.03, (C, 3 * C)).astype(np.float32),
        "b_attn": rng.uniform(-0.03, 0.03, (3 * C,)).astype(np.float32),
        "W_proj": rng.uniform(-0.03, 0.03, (C, C)).astype(np.float32),
        "b_proj": rng.uniform(-0.03, 0.03, (C,)).astype(np.float32),
    }
    out = kernel(**ins)
    print("ran, out shape", out.shape)



# revision 32
# speedup vs baseline: 1.1198x; 1.1198x over previous
"""Causal self-attention (B=4, T=2048, C=1024, H=16, HD=64) on 8 trn2 cores.

Sharding: core = (batch b, head-group g), g in {0,1} covering 8 heads each.
Host sums the two group partials per batch and adds b_eff = b_proj + b_v@W_proj
(b_v passes through softmax exactly since the weights sum to 1).

v5 design (v1 = 291us -> 267us, max rel err 8.4e-3, gate 2e-2):
  - All projections fp8 DoubleRow (~1.8x PE per instr at FD>=512): QKV from
    x8/weights x16, out-proj from y8 = 16*y fp8 pairs vs 16*Wp fp8, /256 on
    the PSUM copy. First 256 tokens' V and first 128 tokens' out-proj go
    through parallel bf16 paths (xb01/wvb, y8b/wpb): short causal rows can't
    average away fp8 quantization (fp8 V and fp8 y are ~3% el-wise, and row 0
    has y = v_0 exactly).
  - S = K^T Q bf16, two heads co-running in 64-row groups (tile_position).
  - Per-kc S PSUM tiles [128, 2(head), 512] double-buffered in 4 banks; one
    exp instruction per kc covers both heads (FD 1024) and alternates
    engines: even kc ACT (true exp), odd kc DVE via Schraudolph
    uint8 = round(A8*s + B8) bitcast fp8e4m3 (HW rounds f32->int; B8
    calibrated; softmax denominators cancel the shared scale wiggle).
    Off-diag exp lands in an SBUF ring pairing 2 kcs -> fp8-DR AV.
  - Diagonal kcs: bf16 tiles (ACT exp on j0/j2, DVE int16-Schraudolph bitcast
    bf16 on j1/j3), one 2x-mode tri-mask TT per kc, per-kc AV with mixed
    dtype lhsT (fp8 vaug8 x bf16 rhs runs at bf16 speed; qc0 reads bf16
    vaugb for accuracy). Ones col = 16.0 makes PSUM row 64 the denominator;
    the 16x V scale cancels in normalize.
  - normalize: sums row copy x(1/16) on ACT -> one reciprocal_approx_fast on
    [1, 2, 512] -> gpsimd partition-broadcast -> TT mult PSUM->y8 fp8 (odd
    heads staged + DMA partition shift); qc0 also writes bf16 y8b cols 0:128.
  - Schedule: x8 n-major, one DMA descriptor per tensor ordered by first use
    (30 small chained DMAs cost ~2.7us each on the sync queue - v3 lost 40us
    to that). QK n0/n1 -> V0-3 -> qc0 ... with V/QK-n2/n3/out-proj groups as
    fillers dispatched inside emit_b (per off-diag pair + 3 per hp boundary)
    to cover exp latency and keep the PE HAM clock-gate warm (dummy
    ldweights after AV groups help too; PE idle >3.4us re-throttles to
    1.2GHz and HW measured 74-206us of throttle when the pipeline bubbled).
  - PSUM budget (8 banks): S pool 2x2 + y pool 2x1 (ype/ypo) + proj 2x1.
"""

import numpy as np

B, T, C, H, HD = 4, 2048, 1024, 16, 64
G = 2              # head groups (tensor parallel)
HG = H // G        # 8 heads per group
GC = HG * HD       # 512 group channels
P = 128
NQC = T // 512     # 4 q-chunks of 512
NKC = T // P       # 16 k-chunks of 128
KO_C = C // P      # 8 contraction chunks for C=1024
KO_G = GC // P     # 4 contraction chunks for GC=512
WSCALE = 16.0      # host-side premultiplier on fp8 weights (stay in normals)

# Schraudolph exp constants (HW rounds on f32->int cast; probed).
A8 = 8 * np.log2(np.e) * 0.125    # uint8 -> fp8e4m3 bitcast ~= exp(s/8)
B8 = 55.60
A16 = 128 * np.log2(np.e) * 0.125  # int16 -> bf16 bitcast ~= exp(s/8)
B16 = 16249.6

_cache = {}


def _build():
    import concourse.bass as bass
    import concourse.tile as tile
    from concourse import bacc, mybir

    f32 = mybir.dt.float32
    bf16 = mybir.dt.bfloat16
    fp8 = mybir.dt.float8e4
    u8 = mybir.dt.uint8
    i16 = mybir.dt.int16
    DR = mybir.MatmulPerfMode.DoubleRow
    EXP = mybir.ActivationFunctionType.Exp
    IDENT = mybir.ActivationFunctionType.Identity
    MULT = mybir.AluOpType.mult
    ADD = mybir.AluOpType.add

    nc = bacc.Bacc(name="csa2")
    x8 = nc.declare_dram_parameter("x8", [P, NQC, KO_C, 512], fp8, isOutput=False)
    wqk = nc.declare_dram_parameter("wqk", [P, 2 * GC // P, KO_C, P], fp8, isOutput=False)
    bqk = nc.declare_dram_parameter("bqk", [P, 2 * GC // P], f32, isOutput=False)
    wv = nc.declare_dram_parameter("wv", [P, KO_C, GC], fp8, isOutput=False)
    wvb = nc.declare_dram_parameter("wvb", [P, KO_C, GC], bf16, isOutput=False)
    xb01 = nc.declare_dram_parameter("xb01", [P, KO_C, 256], bf16, isOutput=False)
    wp = nc.declare_dram_parameter("wp", [P, KO_G, C], fp8, isOutput=False)
    wpb = nc.declare_dram_parameter("wpb", [P, KO_G, C], bf16, isOutput=False)
    mask = nc.declare_dram_parameter("mask", [P, P], bf16, isOutput=False)
    out = nc.declare_dram_parameter("out", [T, C], f32, isOutput=True)

    from contextlib import ExitStack

    with tile.TileContext(nc) as tc, ExitStack() as ctx:
        singles = ctx.enter_context(tc.tile_pool(name="singles", bufs=1))
        pt8p = ctx.enter_context(tc.tile_pool(name="pt8p", bufs=4))
        ptdp = ctx.enter_context(tc.tile_pool(name="ptdp", bufs=4))
        stage = ctx.enter_context(tc.tile_pool(name="stage", bufs=6))
        osb = ctx.enter_context(tc.tile_pool(name="osb", bufs=3))
        # PSUM: spool 2x2 banks + ypool 2x1 + projp 2x1 = 8 banks
        spool = ctx.enter_context(tc.tile_pool(name="spool", bufs=2, space="PSUM"))
        ypool = ctx.enter_context(tc.tile_pool(name="ypool", bufs=2, space="PSUM"))
        projp = ctx.enter_context(tc.tile_pool(name="projp", bufs=2, space="PSUM"))

        # ---- resident tensors ----
        x8_s = singles.tile([P, NQC, KO_C, 512], fp8, tag="x8")
        wqk_s = singles.tile([P, 2 * GC // P, KO_C, P], fp8, tag="wqk")
        wv_s = singles.tile([P, KO_C, GC], fp8, tag="wv")
        wvb_s = singles.tile([P, KO_C, GC], bf16, tag="wvb")
        xb01_s = singles.tile([P, KO_C, 256], bf16, tag="xb01")
        wp_s = singles.tile([P, KO_G, C], fp8, tag="wp")
        QT = singles.tile([P, HG // 2, T], bf16, tag="QT")
        KT = singles.tile([P, HG // 2, T], bf16, tag="KT")
        # off-diag V (fp8, 16x): [P, kc, h, 80]; kc-pair stride 640B (%16 ok);
        # cols 0:64 = 16V, col 64 = 16.0 (denominator), 65:80 pad
        vaug8 = singles.tile([P, NKC, HG, 80], fp8, tag="vaug8")
        # diag V for qc0 only (bf16, 16x): kc 0..3
        vaugb = singles.tile([P, 4, HG, 65], bf16, tag="vaugb")
        # fp8 exp ring: (ring slot, head, kc-of-pair, col)
        pt8r = singles.tile([P, 3, 2, 2, 512], fp8, tag="pt8r")
        y8 = singles.tile([P, KO_G, T], fp8, tag="y8")
        y8b = singles.tile([P, KO_G, P], bf16, tag="y8b")
        wpb_s = singles.tile([P, KO_G, C], bf16, tag="wpb")
        tri = singles.tile([P, P], bf16, tag="tri")
        bqk_s = singles.tile([P, 2 * GC // P], f32, tag="bqk")

        nc.vector.memset(vaug8[:, :, :, 64:65], 16.0)
        nc.vector.memset(vaugb[:, :, :, 64:65], 16.0)

        # ---- input DMA: one descriptor per tensor, emission order = priority
        nc.sync.dma_start(out=x8_s[:, 0, 0:4], in_=x8[:, 0, 0:4])
        nc.sync.dma_start(out=wqk_s[:, 0], in_=wqk[:, 0])
        nc.sync.dma_start(out=x8_s[:, 0, 4:8], in_=x8[:, 0, 4:8])
        nc.sync.dma_start(out=wqk_s[:, 4], in_=wqk[:, 4])
        nc.sync.dma_start(out=wqk_s[:, 1:4], in_=wqk[:, 1:4])
        nc.sync.dma_start(out=wqk_s[:, 5:8], in_=wqk[:, 5:8])
        nc.sync.dma_start(out=bqk_s[:], in_=bqk[:])
        nc.sync.dma_start(out=tri[:], in_=mask[:])
        nc.sync.dma_start(out=x8_s[:, 1], in_=x8[:, 1])
        nc.sync.dma_start(out=wvb_s[:], in_=wvb[:])
        nc.sync.dma_start(out=xb01_s[:], in_=xb01[:])
        nc.sync.dma_start(out=x8_s[:, 2], in_=x8[:, 2])
        nc.sync.dma_start(out=wv_s[:], in_=wv[:])
        nc.sync.dma_start(out=x8_s[:, 3], in_=x8[:, 3])
        nc.sync.dma_start(out=wp_s[:], in_=wp[:])
        nc.sync.dma_start(out=wpb_s[:], in_=wpb[:])

        # ---- QK projection: fp8 DR; ACT copy (scale 1/16 + bias) ----
        def emit_qk_group(m, n):
            acc = projp.tile([P, 512], f32, tag="projp")
            for k2 in range(KO_C // 2):
                nc.tensor.matmul(
                    acc[:],
                    lhsT=wqk_s[:, m, 2 * k2:2 * k2 + 2, :],
                    rhs=x8_s[:, n, 2 * k2:2 * k2 + 2, :],
                    start=(k2 == 0),
                    stop=(k2 == KO_C // 2 - 1),
                    perf_mode=DR,
                )
            dest = QT if m < 4 else KT
            dap = dest[:, m % 4, n * 512:(n + 1) * 512]
            nc.scalar.activation(dap, acc[:], IDENT,
                                 bias=bqk_s[:, m:m + 1], scale=1.0 / WSCALE)

        # ---- V projection: fp8 DR -> PSUM 16*V -> vaug8 (fp8) + vaugb ----
        def emit_v(t):
            n, ts = t // 4, (t % 4) * P
            acc = projp.tile([P, 512], f32, tag="projp")
            if t < 2:
                # bf16 projection for the first 256 tokens: short causal rows
                # can't average away fp8 V error
                for ko in range(KO_C):
                    nc.tensor.matmul(
                        acc[:],
                        lhsT=xb01_s[:, ko, t * P:(t + 1) * P],
                        rhs=wvb_s[:, ko, :],
                        start=(ko == 0),
                        stop=(ko == KO_C - 1),
                    )
            else:
                for k2 in range(KO_C // 2):
                    nc.tensor.matmul(
                        acc[:],
                        lhsT=x8_s[:, n, 2 * k2:2 * k2 + 2, ts:ts + P],
                        rhs=wv_s[:, 2 * k2:2 * k2 + 2, :],
                        start=(k2 == 0),
                        stop=(k2 == KO_C // 2 - 1),
                        perf_mode=DR,
                    )
            acch = acc[:].rearrange("p (h d) -> p h d", h=HG)
            nc.scalar.activation(vaug8[:, t, :, 0:64], acch, IDENT)
            if t < 4:
                nc.vector.tensor_copy(out=vaugb[:, t, :, 0:64], in_=acch)

        # ---- output projection: fp8 DR (lhsT = y8 pairs) ----
        def emit_c(t, tail=False):
            accs = [projp.tile([P, 512], f32, tag="projp", name=f"oacc{_c}") for _c in range(2)]
            if t == 0:
                # bf16 for the first tokens: fp8 y8 quantization (3% of |y|)
                # is worst on short rows where |y| is large
                for ko in range(KO_G):
                    for ch in range(2):
                        nc.tensor.matmul(
                            accs[ch][:],
                            lhsT=y8b[:, ko, :],
                            rhs=wpb_s[:, ko, ch * 512:(ch + 1) * 512],
                            start=(ko == 0),
                            stop=(ko == KO_G - 1),
                        )
            else:
                for kop in range(2):
                    for ch in range(2):
                        nc.tensor.matmul(
                            accs[ch][:],
                            lhsT=y8[:, 2 * kop:2 * kop + 2, t * P:(t + 1) * P],
                            rhs=wp_s[:, 2 * kop:2 * kop + 2, ch * 512:(ch + 1) * 512],
                            start=(kop == 0),
                            stop=(kop == 1),
                            perf_mode=DR,
                        )
            for ch in range(2):
                o = osb.tile([P, 512], f32, tag="osb")
                if ch == 1 and tail:
                    nc.vector.tensor_scalar_mul(o[:], accs[ch][:], 1.0 / 256.0)
                else:
                    nc.scalar.activation(o[:], accs[ch][:], IDENT,
                                         scale=1.0 / 256.0)
                nc.sync.dma_start(
                    out=out[t * P:(t + 1) * P, ch * 512:(ch + 1) * 512],
                    in_=o[:],
                )

        # ---- attention ----
        ring_ctr = [0]

        def emit_b(qc, hp, filler):
            yps = [ypool.tile([P, 512], f32, tag="yp", name=f"yp{_h}")
                   for _h in range(2)]
            first_av = [True, True]
            tribc = tri[:, None, :].to_broadcast((P, 2, P))

            def s_mm(h, kc, spt, qo, w):
                po = 64 * h
                nc.tensor.matmul(
                    spt[:, h, qo:qo + w],
                    lhsT=KT[po:po + 64, hp, kc * P:(kc + 1) * P],
                    rhs=QT[po:po + 64, hp, qc * 512 + qo:qc * 512 + qo + w],
                    start=True, stop=True,
                )

            # --- off-diagonal kcs: per-kc S psum, exp into fp8 pair ring,
            #     DR AV per pair; AV lags one pair so the PE never waits on
            #     exp latency (S of the next pair runs in between) ---
            rs = 0
            pend_av = []

            def flush_av():
                for pr_, rs_ in pend_av:
                    for h in range(2):
                        nc.tensor.matmul(
                            yps[h][0:65, :],
                            lhsT=vaug8[:, 2 * pr_:2 * pr_ + 2, 2 * hp + h, 0:65],
                            rhs=pt8r[:, rs_, h, :, :],
                            start=first_av[h], stop=False,
                            perf_mode=DR,
                        )
                        first_av[h] = False
                pend_av.clear()

            for kc in range(4 * qc):
                pr, jj = divmod(kc, 2)
                if jj == 0:
                    rs = ring_ctr[0] % 3
                    ring_ctr[0] += 1
                spt = spool.tile([P, 2, 512], f32, tag="sp")
                for h in range(2):
                    s_mm(h, kc, spt, 0, 512)
                if jj == 1:
                    flush_av()
                dst = pt8r[:, rs, :, jj, :]
                if kc % 2 == 0:
                    nc.scalar.activation(dst, spt[:], EXP, scale=0.125)
                else:
                    nc.vector.tensor_scalar(
                        out=dst.bitcast(u8), in0=spt[:],
                        scalar1=float(A8), scalar2=float(B8),
                        op0=MULT, op1=ADD,
                    )
                if jj == 1:
                    pend_av.append((pr, rs))
                    filler()
                    nc.tensor.ldweights(tri[:])

            # --- diagonal kcs: per-kc tiles; qc0 in bf16 (accuracy for
            #     short rows, DVE 2x mask), qc>=1 in fp8 (gpsimd mask) ---
            pend_d = []

            def flush_diag():
                for j_, ptd_ in pend_d:
                    kc_ = 4 * qc + j_
                    qo_ = 128 * j_
                    for h in range(2):
                        lhs = (vaugb[:, kc_, 2 * hp + h, :] if qc == 0
                               else vaug8[:, kc_, 2 * hp + h, 0:65])
                        nc.tensor.matmul(
                            yps[h][0:65, qo_:512],
                            lhsT=lhs,
                            rhs=ptd_[:, h, qo_:512],
                            start=first_av[h], stop=(j_ == 3),
                        )
                        first_av[h] = False
                pend_d.clear()

            for j in range(4):
                kc = 4 * qc + j
                qo = 128 * j
                w = 512 - qo
                spt = spool.tile([P, 2, 512], f32, tag="sp")
                for h in range(2):
                    s_mm(h, kc, spt, qo, w)
                if j == 0:
                    flush_av()
                flush_diag()
                ptd = ptdp.tile([P, 2, 512], bf16, tag="ptd")
                if j % 2 == 0:
                    nc.scalar.activation(
                        ptd[:, :, qo:512], spt[:, :, qo:512], EXP, scale=0.125)
                else:
                    nc.vector.tensor_scalar(
                        out=ptd[:, :, qo:512].bitcast(i16), in0=spt[:, :, qo:512],
                        scalar1=float(A16), scalar2=float(B16),
                        op0=MULT, op1=ADD,
                    )
                nc.vector.tensor_tensor(
                    ptd[:, :, qo:qo + P], ptd[:, :, qo:qo + P], tribc, MULT)
                pend_d.append((j, ptd))
                if j % 2 == 1:
                    filler()
                nc.tensor.ldweights(tri[:])
            flush_diag()

            # --- normalize -> y8 (fp8, 16x) ---
            sums = stage.tile([1, 2, 512], f32, tag="sums")
            for h in range(2):
                nc.scalar.activation(sums[:, h, :], yps[h][64:65, :], IDENT,
                                     scale=1.0 / 16.0)
            nc.vector.reciprocal_approx_fast(out=sums[:], in_=sums[:])
            for h in range(2):
                hh = 2 * hp + h
                srep = stage.tile([64, 512], f32, tag="srep")
                nc.gpsimd.partition_broadcast(srep[:], sums[:, h, :])
                dst_ko = hh // 2
                if h == 0:
                    dst = y8[0:64, dst_ko, qc * 512:(qc + 1) * 512]
                    nc.vector.tensor_tensor(dst, yps[h][0:64, :], srep[:], MULT)
                    if qc == 0:
                        nc.vector.tensor_tensor(
                            y8b[0:64, dst_ko, :], yps[h][0:64, 0:P],
                            srep[:, 0:P], MULT)
                else:
                    tmp = stage.tile([64, 512], fp8, tag="ytmp")
                    nc.vector.tensor_tensor(tmp[:], yps[h][0:64, :], srep[:], MULT)
                    nc.sync.dma_start(
                        out=y8[64:128, dst_ko, qc * 512:(qc + 1) * 512],
                        in_=tmp[:])
                    if qc == 0:
                        tmpb = stage.tile([64, P], bf16, tag="ytmpb")
                        nc.vector.tensor_tensor(
                            tmpb[:], yps[h][0:64, 0:P], srep[:, 0:P], MULT)
                        nc.sync.dma_start(
                            out=y8b[64:128, dst_ko, :], in_=tmpb[:])
            filler()
            filler()
            filler()

        # ---- schedule ----
        fillers = []

        def filler():
            if fillers:
                fillers.pop(0)()

        def run_fillers():
            while fillers:
                fillers.pop(0)()

        for m in range(8):
            emit_qk_group(m, 0)
        for m in range(8):
            emit_qk_group(m, 1)
        for t in range(4):
            emit_v(t)
        for t in range(4, 8):
            fillers.append(lambda t=t: emit_v(t))
        for m in range(6):
            fillers.append(lambda m=m: emit_qk_group(m, 2))
        for hp in range(4):
            emit_b(0, hp, filler)
        run_fillers()
        for m in range(6, 8):
            fillers.append(lambda m=m: emit_qk_group(m, 2))
        for t in range(8, 12):
            fillers.append(lambda t=t: emit_v(t))
        fillers.append(lambda: emit_c(0))
        fillers.append(lambda: emit_c(1))
        for hp in range(4):
            emit_b(1, hp, filler)
        run_fillers()
        fillers.append(lambda: emit_c(2))
        fillers.append(lambda: emit_c(3))
        for t in range(12, 16):
            fillers.append(lambda t=t: emit_v(t))
        fillers.append(lambda: emit_qk_group(0, 3))
        fillers.append(lambda: emit_qk_group(4, 3))
        fillers.append(lambda: emit_c(4))
        fillers.append(lambda: emit_c(5))
        for hp in range(4):
            emit_b(2, hp, filler)
        run_fillers()
        for hp in range(1, 4):
            fillers.append(lambda m=hp: emit_qk_group(m, 3))
            fillers.append(lambda m=4 + hp: emit_qk_group(m, 3))
            fillers.append(lambda t=5 + hp: emit_c(t))
        fillers.append(lambda: emit_c(9))
        fillers.append(lambda: emit_c(10))
        fillers.append(lambda: emit_c(11))
        for hp in range(4):
            emit_b(3, hp, filler)
        run_fillers()
        for t in range(12, 16):
            emit_c(t, tail=True)
    nc.finalize()
    return nc


def _get_nc():
    if "nc" not in _cache:
        _cache["nc"] = _build()
    return _cache["nc"]


def _prep_inputs(x, W_attn, b_attn, W_proj):
    import ml_dtypes

    bfloat16 = ml_dtypes.bfloat16
    f8 = ml_dtypes.float8_e4m3
    x = np.ascontiguousarray(np.asarray(x, np.float32))
    W_attn = np.asarray(W_attn, np.float32)
    b_attn = np.asarray(b_attn, np.float32)
    W_proj = np.asarray(W_proj, np.float32)
    mask = (np.arange(P)[:, None] <= np.arange(P)[None, :]).astype(np.float32)
    in_maps = []
    for b in range(B):
        xT = x[b].T  # [C, T]
        x8b = np.ascontiguousarray(
            xT.reshape(KO_C, P, NQC, 512).transpose(1, 2, 0, 3))
        x8b = np.clip(x8b, -240, 240).astype(f8)
        for g in range(G):
            qs, ks, vs = g * GC, C + g * GC, 2 * C + g * GC
            w2 = np.concatenate([W_attn[:, qs:qs + GC], W_attn[:, ks:ks + GC]], 1)
            in_maps.append({
                "x8": x8b,
                "wqk": np.ascontiguousarray(
                    (w2 * WSCALE).reshape(KO_C, P, 2 * GC // P, P)
                    .transpose(1, 2, 0, 3)).astype(f8),
                "bqk": np.ascontiguousarray(
                    np.concatenate([b_attn[qs:qs + GC], b_attn[ks:ks + GC]])
                    .reshape(2 * GC // P, P).T),
                "wv": np.ascontiguousarray(
                    (W_attn[:, vs:vs + GC] * WSCALE).reshape(KO_C, P, GC)
                    .transpose(1, 0, 2)).astype(f8),
                "wvb": np.ascontiguousarray(
                    (W_attn[:, vs:vs + GC] * WSCALE).reshape(KO_C, P, GC)
                    .transpose(1, 0, 2)).astype(bfloat16),
                "xb01": np.ascontiguousarray(
                    xT[:, 0:256].reshape(KO_C, P, 256)
                    .transpose(1, 0, 2)).astype(bfloat16),
                "wp": np.ascontiguousarray(
                    (W_proj[g * GC:(g + 1) * GC, :] * WSCALE).reshape(KO_G, P, C)
                    .transpose(1, 0, 2)).astype(f8),
                "wpb": np.ascontiguousarray(
                    (W_proj[g * GC:(g + 1) * GC, :] * WSCALE).reshape(KO_G, P, C)
                    .transpose(1, 0, 2)).astype(bfloat16),
                "mask": mask.astype(bfloat16),
            })
    return in_maps


def _run(inputs, trace=False):
    from concourse.bass_utils import run_bass_kernel_spmd

    nc = _get_nc()
    in_maps = _prep_inputs(
        inputs["x"], inputs["W_attn"], inputs["b_attn"], inputs["W_proj"]
    )
    res = run_bass_kernel_spmd(nc, in_maps, list(range(B * G)), trace=trace)
    W_proj_f = np.asarray(inputs["W_proj"], np.float32)
    b_attn_f = np.asarray(inputs["b_attn"], np.float32)
    b_eff = (np.asarray(inputs["b_proj"], np.float32)
             + b_attn_f[2 * C:] @ W_proj_f)
    outs = [
        res.results[2 * b]["out"] + res.results[2 * b + 1]["out"] + b_eff
        for b in range(B)
    ]
    return np.stack(outs).astype(np.float32), res


def kernel(**inputs):
    return _run(inputs, trace=False)[0]


if __name__ == "__main__":
    rng = np.random.default_rng(0)
    ins = {
        "x": rng.standard_normal((B, T, C), np.float32),
        "W_attn": rng.uniform(-0.03, 0.03, (C, 3 * C)).astype(np.float32),
        "b_attn": rng.uniform(-0.03, 0.03, (3 * C,)).astype(np.float32),
        "W_proj": rng.uniform(-0.03, 0.03, (C, C)).astype(np.float32),
        "b_proj": rng.uniform(-0.03, 0.03, (C,)).astype(np.float32),
    }
    out = kernel(**ins)
    print("ran, out shape", out.shape)
